# revision 36
# baseline (speedup 1.0000x reference)
"""Self-contained Trainium2 Bass kernel for nn_Attention_11836929868027.

Causal GQA attention prefill (B=2, T=1024, D=4096, 32 q heads / 8 kv heads,
head_dim 128) with per-head RMSNorm on q/k, RoPE, empty kv cache.

Sharding: tensor-parallel over kv-head groups across 8 NeuronCores. Core c
owns kv head c and q heads 4c..4c+3. Each core computes its heads'
projections, attention and a partial o_proj over the full emb_dim; the host
sums the 8 fp32 partials.

Compute dtype: bf16 matmul inputs with fp32 PSUM accumulation (validated
offline: scale-relative absmax err ~4e-3 vs the fp32 reference).
"""

import math

import numpy as np
import ml_dtypes

BF = ml_dtypes.bfloat16

B, T, S = 2, 1024, 2048
D, N, KH, H = 4096, 32, 8, 128
G = N // KH          # 4 q heads per kv head / core
BT = B * T           # 2048 tokens
E = G * H            # 512 q columns per core
DC = D // 128        # 32 contraction chunks
NTC = BT // 128      # 16 token chunks
NQ = BT // 512       # 4 token quarters
EPS = 1e-6
ROPE_THETA = 1e6
NCORES = 8

_CACHE = {}


def _build():
    import concourse.bass as bass
    import concourse.mybir as mybir
    import concourse.tile as tile
    from concourse import bacc
    from concourse.masks import make_identity

    fp32 = mybir.dt.float32
    bf16 = mybir.dt.bfloat16
    MUL = mybir.AluOpType.mult
    SUB = mybir.AluOpType.subtract
    ADD = mybir.AluOpType.add
    AF = mybir.ActivationFunctionType

    nc = bacc.Bacc("TRN2", target_bir_lowering=False, num_devices=NCORES)

    xq_d = nc.declare_dram_parameter("xq", [NQ, 128, DC, 512], bf16, False)
    wqkv_d = nc.declare_dram_parameter("wqkv", [128, DC, E + 2 * H], bf16, False)
    wo_d = nc.declare_dram_parameter("wo", [128, G, D], bf16, False)
    cos_d = nc.declare_dram_parameter("cosq", [128, NTC, 64], fp32, False)
    sin_d = nc.declare_dram_parameter("sinq", [128, NTC, 64], fp32, False)
    qsc_d = nc.declare_dram_parameter("qscale", [128, H], fp32, False)
    ksc_d = nc.declare_dram_parameter("kscale", [128, H], fp32, False)
    mask_d = nc.declare_dram_parameter("maskT", [128, 1024], bf16, False)
    out_d = nc.declare_dram_parameter("out", [BT, D], fp32, True)

    inv_sqrt_h = float(1.0 / math.sqrt(H))

    with tile.TileContext(nc) as tc:
        with (
            tc.tile_pool(name="persist", bufs=1) as pp,
            tc.tile_pool(name="ps", bufs=8, space="PSUM") as ps,
            tc.tile_pool(name="rows", bufs=4) as rows,
            tc.tile_pool(name="qkrp", bufs=3) as qkrp,
        ):
            # ---- persistent SBUF tensors ----
            QT_sb = pp.tile([128, G, BT], bf16, name="QT_sb")
            KT_sb = pp.tile([128, BT], bf16, name="KT_sb")
            V_sb = pp.tile([128, NTC, H], bf16, name="V_sb")
            OT_sb = pp.tile([128, G, BT], bf16, name="OT_sb")
            cos_sb = pp.tile([128, NTC, 64], fp32, name="cos_sb")
            sin_sb = pp.tile([128, NTC, 64], fp32, name="sin_sb")
            qsc_sb = pp.tile([128, H], fp32, name="qsc_sb")
            ksc_sb = pp.tile([128, H], fp32, name="ksc_sb")
            mask_sb = pp.tile([128, 1024], bf16, name="mask_sb")
            ones_bf = pp.tile([128, 1], bf16, name="ones_bf")
            ones_f32 = pp.tile([1, 128], fp32, name="ones_f32")
            ident = pp.tile([128, 128], bf16, name="ident")
            eps_sb = pp.tile([128, 1], fp32, name="eps_sb")

            # const DMAs are deferred until after the first weight/act
            # slices so the first matmuls' data is at the front of the
            # DMA queues
            const_dmas = [
                (cos_sb, cos_d), (sin_sb, sin_d), (qsc_sb, qsc_d),
                (ksc_sb, ksc_d), (mask_sb, mask_d),
            ]
            nc.vector.memset(ones_bf[:], 1.0)
            nc.vector.memset(ones_f32[:], 1.0)
            nc.vector.memset(eps_sb[:], EPS)
            make_identity(nc, ident[:])

            pending = []  # deferred PE transposes: (src_bf16_tile, dst_ap)

            def flush_pending():
                for src, dst_ap in pending:
                    tp = ps.tile([128, 512], bf16, name="tp_ps", tag="ps")
                    nc.tensor.transpose(tp[:, :128], src, ident[:])
                    nc.vector.tensor_copy(dst_ap, tp[:, :128])
                pending.clear()

            # ================= Phase 1: QKV projection =================
            with (
                tc.tile_pool(name="p1w", bufs=1) as p1w,
                tc.tile_pool(name="p1x", bufs=2) as p1x,
                tc.tile_pool(name="p1t", bufs=3) as p1t,
            ):
                wqkv_sb = p1w.tile([128, DC, E + 2 * H], bf16, name="wqkv_sb")
                nc.sync.dma_start(
                    out=wqkv_sb[:, 0:1, :], in_=wqkv_d[:, 0:1, :]
                )
                nc.sync.dma_start(
                    out=wqkv_sb[:, 1:2, :], in_=wqkv_d[:, 1:2, :]
                )
                nc.sync.dma_start(
                    out=wqkv_sb[:, 2:8, :], in_=wqkv_d[:, 2:8, :]
                )

                for q in range(NQ):
                    xq_sb = p1x.tile([128, DC, 512], bf16, name="xq_sb", tag="xq")
                    if q == 0:
                        nc.sync.dma_start(
                            out=xq_sb[:, 0:1, :], in_=xq_d[q, :, 0:1, :]
                        )
                        nc.sync.dma_start(
                            out=xq_sb[:, 1:2, :], in_=xq_d[q, :, 1:2, :]
                        )
                    for k in range(4):
                        lo = 2 if (q == 0 and k == 0) else 8 * k
                        nc.sync.dma_start(
                            out=xq_sb[:, lo:8 * (k + 1), :],
                            in_=xq_d[q, :, lo:8 * (k + 1), :],
                        )
                        if q == 0 and k == 0:
                            for dst, src in const_dmas:
                                nc.sync.dma_start(out=dst[:], in_=src[:])
                            for kw in range(1, 4):
                                nc.sync.dma_start(
                                    out=wqkv_sb[:, 8 * kw:8 * (kw + 1), :],
                                    in_=wqkv_d[:, 8 * kw:8 * (kw + 1), :],
                                )
                    for tl in range(4):
                        tci = q * 4 + tl
                        psA = ps.tile([128, 512], fp32, name="psA", tag="ps")
                        psB = ps.tile([128, 512], fp32, name="psB", tag="ps")
                        for d in range(DC):
                            lhs = xq_sb[:, d, tl * 128:(tl + 1) * 128]
                            nc.tensor.matmul(
                                psA[:], lhs, wqkv_sb[:, d, 0:E],
                                start=(d == 0), stop=(d == DC - 1),
                            )
                            nc.tensor.matmul(
                                psB[:, 0:2 * H], lhs, wqkv_sb[:, d, E:E + 2 * H],
                                start=(d == 0), stop=(d == DC - 1),
                            )
                        # transposes of the previous chunk go here so PE
                        # doesn't stall on this chunk's postprocess
                        flush_pending()

                        # ---- norm + rope for the 4 q heads (batched) ----
                        z2q = p1t.tile([128, E], fp32, name="z2q", tag="z2q")
                        nc.scalar.activation(z2q[:], psA[:], AF.Square)
                        ssq = rows.tile([128, 8], fp32, name="ssq", tag="ssq")
                        nc.vector.reduce_sum(
                            ssq[:, 0:4], z2q[:].rearrange("p (g h) -> p g h", g=G),
                            axis=mybir.AxisListType.X,
                        )
                        # k head: cols 0:128 of psB
                        z2k = p1t.tile([128, H], fp32, name="z2k", tag="z2k")
                        nc.scalar.activation(z2k[:], psB[:, 0:H], AF.Square)
                        nc.vector.reduce_sum(
                            ssq[:, 4:5], z2k[:], axis=mybir.AxisListType.X,
                        )
                        # rstd = 1/sqrt(ssq/H + eps)
                        nc.scalar.activation(
                            ssq[:, 0:5], ssq[:, 0:5], AF.Sqrt,
                            bias=eps_sb[:], scale=float(1.0 / H),
                        )
                        rstd = rows.tile([128, 8], fp32, name="rstd", tag="rstd")
                        nc.vector.reciprocal(rstd[:, 0:5], ssq[:, 0:5])

                        # qkn[:, 0:4] = q heads, qkn[:, 4] = k head (normalized)
                        qkn = p1t.tile([128, G + 1, H], fp32, name="qkn", tag="qkn")
                        nc.vector.tensor_tensor(
                            qkn[:, 0:G, :],
                            psA[:].rearrange("p (g h) -> p g h", g=G),
                            rstd[:, 0:4][:, :, None].to_broadcast((128, G, H)),
                            MUL,
                        )
                        nc.vector.tensor_tensor(
                            qkn[:, 0:G, :], qkn[:, 0:G, :],
                            qsc_sb[:][:, None, :].to_broadcast((128, G, H)),
                            MUL,
                        )
                        nc.vector.scalar_tensor_tensor(
                            qkn[:, G, :], psB[:, 0:H], rstd[:, 4:5], ksc_sb[:],
                            MUL, MUL,
                        )

                        # rope (free-axis halves) on all 5 heads, outputs bf16
                        cs = cos_sb[:, tci, :]
                        sn = sin_sb[:, tci, :]
                        csg = cs[:, None, :].to_broadcast((128, G + 1, 64))
                        sng = sn[:, None, :].to_broadcast((128, G + 1, 64))
                        qkr = qkrp.tile([128, G + 1, H], bf16, name="qkr", tag="qkr")
                        t1 = p1t.tile([128, G + 1, 64], fp32, name="t1", tag="t1")
                        t2 = p1t.tile([128, G + 1, 64], fp32, name="t2", tag="t2")
                        nc.vector.tensor_tensor(t1[:], qkn[:, :, 0:64], csg, MUL)
                        nc.vector.tensor_tensor(t2[:], qkn[:, :, 64:128], sng, MUL)
                        nc.vector.tensor_tensor(qkr[:, :, 0:64], t1[:], t2[:], SUB)
                        nc.vector.tensor_tensor(t1[:], qkn[:, :, 64:128], csg, MUL)
                        nc.vector.tensor_tensor(t2[:], qkn[:, :, 0:64], sng, MUL)
                        nc.vector.tensor_tensor(qkr[:, :, 64:128], t1[:], t2[:], ADD)

                        # V: plain copy/cast into [s, h'] layout
                        nc.vector.tensor_copy(V_sb[:, tci, :], psB[:, H:2 * H])

                        col = tci * 128
                        for g in range(G):
                            pending.append(
                                (qkr[:, g, :], QT_sb[:, g, col:col + 128])
                            )
                        pending.append((qkr[:, G, :], KT_sb[:, col:col + 128]))
                # the final chunk's transposes are NOT flushed here — they
                # are emitted after the first attention section's logits so
                # the PE isn't stalled on the last rope chain

            # ================= Phase 2: attention =================
            with (
                tc.tile_pool(name="p2e", bufs=14) as p2e,
                tc.tile_pool(name="p2t", bufs=3) as p2t,
                tc.tile_pool(name="p3w", bufs=1) as p3w,
                tc.tile_pool(name="p3o", bufs=4) as p3o,
                tc.tile_pool(name="bcd", bufs=4, space="DRAM") as bcd,
            ):
                wo_sb = p3w.tile([128, G, D], bf16, name="wo_sb")
                for k in range(2):
                    nc.sync.dma_start(
                        out=wo_sb[:, 2 * k:2 * (k + 1), :],
                        in_=wo_d[:, 2 * k:2 * (k + 1), :],
                    )

                def emit_p3_half(tci, half, alt):
                    tcol = tci * 128
                    pso = []
                    for dh in range(4 * half, 4 * half + 4):
                        p = ps.tile([128, 512], fp32, name="pso", tag="ps")
                        pso.append((dh, p))
                    for g in range(G):
                        for dh, p in pso:
                            nc.tensor.matmul(
                                p[:],
                                OT_sb[:, g, tcol:tcol + 128],
                                wo_sb[:, g, dh * 512:(dh + 1) * 512],
                                start=(g == 0), stop=(g == G - 1),
                            )
                    for j, (dh, p) in enumerate(pso):
                        ob = p3o.tile([128, 512], fp32, name="ob", tag="ob")
                        if (alt + j) % 2 == 0:
                            nc.vector.tensor_copy(ob[:], p[:])
                        else:
                            nc.scalar.copy(ob[:], p[:])
                        nc.sync.dma_start(
                            out=out_d[tcol:tcol + 128, dh * 512:(dh + 1) * 512],
                            in_=ob[:],
                        )

                p3_queue = [
                    (tci, half) for tci in range(NTC) for half in range(2)
                ]
                p3_done = 0

                for b in range(B):
                    for g in range(G):
                        # pipeline both t0 halves: emit all 12 logits+exp
                        # first so the ACT exp stream runs ahead of the AVs
                        all_eTs = []
                        for t0 in (0, 512):
                            col0 = b * T + t0
                            nS = (t0 + 512) // 128
                            eTs = []
                            for c in range(nS):
                                # columns t_local < 128c - t0 are fully masked
                                # by causality — skip computing them entirely
                                off = max(0, 128 * c - t0)
                                Lp = ps.tile(
                                    [128, 512], fp32, name="Lp", tag="ps"
                                )
                                nc.tensor.matmul(
                                    Lp[:, off:512],
                                    KT_sb[:, b * T + c * 128:b * T + (c + 1) * 128],
                                    QT_sb[:, g, col0 + off:col0 + 512],
                                    start=True, stop=True,
                                )
                                eT = p2e.tile(
                                    [128, 512], bf16, name="eT", tag="eT"
                                )
                                nc.scalar.activation(
                                    eT[:, off:512], Lp[:, off:512],
                                    AF.Exp, scale=inv_sqrt_h,
                                )
                                if 128 * c + 127 > t0:  # diagonal-crossing block
                                    u0 = 512 + t0 - 128 * c
                                    nc.vector.tensor_tensor(
                                        eT[:, off:512], eT[:, off:512],
                                        mask_sb[:, u0 + off:u0 + 512], MUL,
                                    )
                                eTs.append((eT, off))
                            all_eTs.append((eTs, col0, nS))

                        # deferred final-chunk transposes (only non-empty on
                        # the very first section, which doesn't depend on them)
                        if pending:
                            flush_pending()

                        # batch-0 o_proj chunks go HERE — between the logits
                        # and the AVs — exactly where the PE would otherwise
                        # wait for the exp stream to catch up
                        if b == 1:
                            for _ in range(4):
                                tci_h = p3_queue[p3_done]
                                emit_p3_half(tci_h[0], tci_h[1], p3_done)
                                p3_done += 1

                        for eTs, col0, nS in all_eTs:
                            t0b = col0 - b * T
                            OTp = ps.tile([128, 512], fp32, name="OTp", tag="ps")
                            Sp = ps.tile([128, 512], fp32, name="Sp", tag="ps")
                            for c in range(nS):
                                sc = b * (T // 128) + c
                                eT, off = eTs[c]
                                nc.tensor.matmul(
                                    OTp[:, off:512], V_sb[:, sc, :],
                                    eT[:, off:512],
                                    start=(c == 0), stop=(c == nS - 1),
                                )
                                nc.tensor.matmul(
                                    Sp[:1, off:512], ones_bf[:], eT[:, off:512],
                                    start=(c == 0), stop=(c == nS - 1),
                                )
                            recip = rows.tile(
                                [1, 512], fp32, name="recip", tag="recip"
                            )
                            nc.vector.reciprocal(recip[:], Sp[:1, :])
                            # softmax normalization with the PE fully out of
                            # the path: bounce the recip row through DRAM and
                            # DMA it back partition-broadcast (DRAM-source
                            # zero-step partition APs are legal, cf. the
                            # groupnorm bias broadcast)
                            bd = bcd.tile([1, 512], fp32, name="bd", tag="bd")
                            nc.sync.dma_start(out=bd[:], in_=recip[:])
                            bcs = p2t.tile([128, 512], fp32, name="bcs", tag="bcs")
                            nc.gpsimd.dma_start(
                                out=bcs[:], in_=bd[:].to_broadcast((128, 512))
                            )
                            nc.vector.tensor_tensor(
                                OT_sb[:, g, col0:col0 + 512], OTp[:], bcs[:], MUL
                            )

                # ================= Phase 3: remaining o_proj =================
                while p3_done < len(p3_queue):
                    tci_h = p3_queue[p3_done]
                    emit_p3_half(tci_h[0], tci_h[1], p3_done)
                    p3_done += 1

    nc.compile()
    return nc


def _prep_inputs(x, wq, wk, wv, wo, q_scale, k_scale, segment_ids):
    """Host-side shard prep. Returns in_maps for the 8 cores."""
    x2 = np.ascontiguousarray(np.asarray(x, dtype=np.float32).reshape(BT, D))
    xT = x2.T.astype(BF)                                   # [D, BT]
    xq = np.ascontiguousarray(
        xT.reshape(DC, 128, NQ, 512).transpose(2, 1, 0, 3)
    )                                                      # [NQ,128,DC,512]

    seg = np.asarray(segment_ids)
    first = np.argmax(seg, axis=1)
    pos = np.where(
        seg != 0, np.arange(T, dtype=np.int64)[None, :] - first[:, None], 2 ** 30
    )
    fraction = np.arange(0, H, 2, dtype=np.float64) / H
    inv_freq = 1.0 / (ROPE_THETA ** fraction)
    sinus = pos.reshape(-1).astype(np.float64)[:, None] * inv_freq[None, :]
    cosf = np.cos(sinus).astype(np.float32)                # [BT, 64]
    sinf = np.sin(sinus).astype(np.float32)
    cosq = np.ascontiguousarray(cosf.reshape(NTC, 128, 64).transpose(1, 0, 2))
    sinq = np.ascontiguousarray(sinf.reshape(NTC, 128, 64).transpose(1, 0, 2))

    qsc = np.ascontiguousarray(
        np.tile(np.asarray(q_scale, np.float32)[None, :], (128, 1))
    )
    ksc = np.ascontiguousarray(
        np.tile(np.asarray(k_scale, np.float32)[None, :], (128, 1))
    )

    su = np.arange(128)[:, None] <= (np.arange(1024)[None, :] - 512)
    maskT = su.astype(BF)                                  # [128, 1024]

    wq2 = np.asarray(wq, np.float32).reshape(D, N * H)
    wk2 = np.asarray(wk, np.float32).reshape(D, KH * H)
    wv2 = np.asarray(wv, np.float32).reshape(D, KH * H)
    wo2 = np.asarray(wo, np.float32)                       # [N, H, D]

    in_maps = []
    for c in range(NCORES):
        wqkv = np.concatenate(
            [
                wq2[:, c * E:(c + 1) * E],
                wk2[:, c * H:(c + 1) * H],
                wv2[:, c * H:(c + 1) * H],
            ],
            axis=1,
        ).astype(BF)                                       # [D, 768]
        wqkvt = np.ascontiguousarray(
            wqkv.reshape(DC, 128, E + 2 * H).transpose(1, 0, 2)
        )                                                  # [128, DC, 768]
        woc = wo2[c * G:(c + 1) * G].astype(BF)            # [G, H, D]
        wot = np.ascontiguousarray(woc.transpose(1, 0, 2))  # [128, G, D]
        in_maps.append(
            {
                "xq": xq,
                "wqkv": wqkvt,
                "wo": wot,
                "cosq": cosq,
                "sinq": sinq,
                "qscale": qsc,
                "kscale": ksc,
                "maskT": maskT,
            }
        )
    return in_maps


def kernel(x, wq, wk, wv, wo, q_scale, k_scale, k_cache, v_cache,
           segment_ids, num_right_pads=0, **_unused):
    from concourse.bass_utils import run_bass_kernel_spmd

    if "nc" not in _CACHE:
        _CACHE["nc"] = _build()
    nc = _CACHE["nc"]

    in_maps = _prep_inputs(x, wq, wk, wv, wo, q_scale, k_scale, segment_ids)
    res = run_bass_kernel_spmd(nc, in_maps, core_ids=list(range(NCORES)))
    total = np.zeros((BT, D), np.float32)
    for c in range(NCORES):
        total += np.asarray(res.results[c]["out"], dtype=np.float32)
    return total.reshape(B, T, D)


# revision 39
# speedup vs baseline: 1.0136x; 1.0136x over previous
"""Self-contained Trainium2 Bass kernel for nn_Attention_11836929868027.

Causal GQA attention prefill (B=2, T=1024, D=4096, 32 q heads / 8 kv heads,
head_dim 128) with per-head RMSNorm on q/k, RoPE, empty kv cache.

Sharding: tensor-parallel over kv-head groups across 8 NeuronCores. Core c
owns kv head c and q heads 4c..4c+3. Each core computes its heads'
projections, attention and a partial o_proj over the full emb_dim; the host
sums the 8 fp32 partials.

Compute dtype: bf16 matmul inputs with fp32 PSUM accumulation (validated
offline: scale-relative absmax err ~4e-3 vs the fp32 reference).
"""

import math

import numpy as np
import ml_dtypes

BF = ml_dtypes.bfloat16

B, T, S = 2, 1024, 2048
D, N, KH, H = 4096, 32, 8, 128
G = N // KH          # 4 q heads per kv head / core
BT = B * T           # 2048 tokens
E = G * H            # 512 q columns per core
DC = D // 128        # 32 contraction chunks
NTC = BT // 128      # 16 token chunks
NQ = BT // 512       # 4 token quarters
EPS = 1e-6
ROPE_THETA = 1e6
NCORES = 8

_CACHE = {}


def _build():
    import concourse.bass as bass
    import concourse.mybir as mybir
    import concourse.tile as tile
    from concourse import bacc
    from concourse.masks import make_identity

    fp32 = mybir.dt.float32
    bf16 = mybir.dt.bfloat16
    MUL = mybir.AluOpType.mult
    SUB = mybir.AluOpType.subtract
    ADD = mybir.AluOpType.add
    AF = mybir.ActivationFunctionType

    nc = bacc.Bacc("TRN2", target_bir_lowering=False, num_devices=NCORES)

    xq_d = nc.declare_dram_parameter("xq", [NQ, 128, DC, 512], bf16, False)
    wqkv_d = nc.declare_dram_parameter("wqkv", [128, DC, E + 2 * H], bf16, False)
    wo_d = nc.declare_dram_parameter("wo", [128, G, D], bf16, False)
    cos_d = nc.declare_dram_parameter("cosq", [128, NTC, 64], fp32, False)
    sin_d = nc.declare_dram_parameter("sinq", [128, NTC, 64], fp32, False)
    qsc_d = nc.declare_dram_parameter("qscale", [128, H], fp32, False)
    ksc_d = nc.declare_dram_parameter("kscale", [128, H], fp32, False)
    mask_d = nc.declare_dram_parameter("maskT", [128, 1024], bf16, False)
    out_d = nc.declare_dram_parameter("out", [BT, D], fp32, True)

    inv_sqrt_h = float(1.0 / math.sqrt(H))

    with tile.TileContext(nc) as tc:
        with (
            tc.tile_pool(name="persist", bufs=1) as pp,
            tc.tile_pool(name="ps", bufs=8, space="PSUM") as ps,
            tc.tile_pool(name="rows", bufs=4) as rows,
        ):
            # ---- persistent SBUF tensors ----
            QT_sb = pp.tile([128, G, BT], bf16, name="QT_sb")
            KT_sb = pp.tile([128, BT], bf16, name="KT_sb")
            V_sb = pp.tile([128, NTC, H], bf16, name="V_sb")
            OT_sb = pp.tile([128, G, BT], bf16, name="OT_sb")
            cos_sb = pp.tile([128, NTC, 64], fp32, name="cos_sb")
            sin_sb = pp.tile([128, NTC, 64], fp32, name="sin_sb")
            qsc_sb = pp.tile([128, H], fp32, name="qsc_sb")
            ksc_sb = pp.tile([128, H], fp32, name="ksc_sb")
            mask_sb = pp.tile([128, 1024], bf16, name="mask_sb")
            ones_bf = pp.tile([128, 1], bf16, name="ones_bf")
            ones_f32 = pp.tile([1, 128], fp32, name="ones_f32")
            ident = pp.tile([128, 128], bf16, name="ident")
            eps_sb = pp.tile([128, 1], fp32, name="eps_sb")

            # const DMAs are deferred until after the first weight/act
            # slices so the first matmuls' data is at the front of the
            # DMA queues
            const_dmas = [
                (cos_sb, cos_d), (sin_sb, sin_d), (qsc_sb, qsc_d),
                (ksc_sb, ksc_d), (mask_sb, mask_d),
            ]
            nc.vector.memset(ones_bf[:], 1.0)
            nc.vector.memset(ones_f32[:], 1.0)
            nc.vector.memset(eps_sb[:], EPS)
            make_identity(nc, ident[:])

            # ================= Phase 1: QKV projection =================
            with (
                tc.tile_pool(name="p1w", bufs=1) as p1w,
                tc.tile_pool(name="p1x", bufs=2) as p1x,
                tc.tile_pool(name="p1t", bufs=3) as p1t,
            ):
                wqkv_sb = p1w.tile([128, DC, E + 2 * H], bf16, name="wqkv_sb")
                nc.sync.dma_start(
                    out=wqkv_sb[:, 0:1, :], in_=wqkv_d[:, 0:1, :]
                )
                nc.sync.dma_start(
                    out=wqkv_sb[:, 1:2, :], in_=wqkv_d[:, 1:2, :]
                )
                nc.sync.dma_start(
                    out=wqkv_sb[:, 2:8, :], in_=wqkv_d[:, 2:8, :]
                )

                pending = []  # deferred PE transposes: (src_bf16_tile, g_or_None, tc_idx)

                def flush_pending():
                    for src, dst_ap in pending:
                        tp = ps.tile([128, 512], bf16, name="tp_ps", tag="ps")
                        nc.tensor.transpose(tp[:, :128], src, ident[:])
                        nc.vector.tensor_copy(dst_ap, tp[:, :128])
                    pending.clear()

                for q in range(NQ):
                    xq_sb = p1x.tile([128, DC, 512], bf16, name="xq_sb", tag="xq")
                    if q == 0:
                        nc.sync.dma_start(
                            out=xq_sb[:, 0:1, :], in_=xq_d[q, :, 0:1, :]
                        )
                        nc.sync.dma_start(
                            out=xq_sb[:, 1:2, :], in_=xq_d[q, :, 1:2, :]
                        )
                    for k in range(4):
                        lo = 2 if (q == 0 and k == 0) else 8 * k
                        nc.sync.dma_start(
                            out=xq_sb[:, lo:8 * (k + 1), :],
                            in_=xq_d[q, :, lo:8 * (k + 1), :],
                        )
                        if q == 0 and k == 0:
                            for dst, src in const_dmas:
                                nc.sync.dma_start(out=dst[:], in_=src[:])
                            for kw in range(1, 4):
                                nc.sync.dma_start(
                                    out=wqkv_sb[:, 8 * kw:8 * (kw + 1), :],
                                    in_=wqkv_d[:, 8 * kw:8 * (kw + 1), :],
                                )
                    for tl in range(4):
                        tci = q * 4 + tl
                        psA = ps.tile([128, 512], fp32, name="psA", tag="ps")
                        psB = ps.tile([128, 512], fp32, name="psB", tag="ps")
                        for d in range(DC):
                            lhs = xq_sb[:, d, tl * 128:(tl + 1) * 128]
                            nc.tensor.matmul(
                                psA[:], lhs, wqkv_sb[:, d, 0:E],
                                start=(d == 0), stop=(d == DC - 1),
                            )
                            nc.tensor.matmul(
                                psB[:, 0:2 * H], lhs, wqkv_sb[:, d, E:E + 2 * H],
                                start=(d == 0), stop=(d == DC - 1),
                            )
                        # transposes of the previous chunk go here so PE
                        # doesn't stall on this chunk's postprocess
                        flush_pending()

                        # ---- norm + rope for the 4 q heads (batched) ----
                        z2q = p1t.tile([128, E], fp32, name="z2q", tag="z2q")
                        nc.scalar.activation(z2q[:], psA[:], AF.Square)
                        ssq = rows.tile([128, 8], fp32, name="ssq", tag="ssq")
                        nc.vector.reduce_sum(
                            ssq[:, 0:4], z2q[:].rearrange("p (g h) -> p g h", g=G),
                            axis=mybir.AxisListType.X,
                        )
                        # k head: cols 0:128 of psB
                        z2k = p1t.tile([128, H], fp32, name="z2k", tag="z2k")
                        nc.scalar.activation(z2k[:], psB[:, 0:H], AF.Square)
                        nc.vector.reduce_sum(
                            ssq[:, 4:5], z2k[:], axis=mybir.AxisListType.X,
                        )
                        # rstd = 1/sqrt(ssq/H + eps)
                        nc.scalar.activation(
                            ssq[:, 0:5], ssq[:, 0:5], AF.Sqrt,
                            bias=eps_sb[:], scale=float(1.0 / H),
                        )
                        rstd = rows.tile([128, 8], fp32, name="rstd", tag="rstd")
                        nc.vector.reciprocal(rstd[:, 0:5], ssq[:, 0:5])

                        # qkn[:, 0:4] = q heads, qkn[:, 4] = k head (normalized)
                        qkn = p1t.tile([128, G + 1, H], fp32, name="qkn", tag="qkn")
                        nc.vector.tensor_tensor(
                            qkn[:, 0:G, :],
                            psA[:].rearrange("p (g h) -> p g h", g=G),
                            rstd[:, 0:4][:, :, None].to_broadcast((128, G, H)),
                            MUL,
                        )
                        nc.vector.tensor_tensor(
                            qkn[:, 0:G, :], qkn[:, 0:G, :],
                            qsc_sb[:][:, None, :].to_broadcast((128, G, H)),
                            MUL,
                        )
                        nc.vector.scalar_tensor_tensor(
                            qkn[:, G, :], psB[:, 0:H], rstd[:, 4:5], ksc_sb[:],
                            MUL, MUL,
                        )

                        # rope (free-axis halves) on all 5 heads, outputs bf16
                        cs = cos_sb[:, tci, :]
                        sn = sin_sb[:, tci, :]
                        csg = cs[:, None, :].to_broadcast((128, G + 1, 64))
                        sng = sn[:, None, :].to_broadcast((128, G + 1, 64))
                        qkr = p1t.tile([128, G + 1, H], bf16, name="qkr", tag="qkr")
                        t1 = p1t.tile([128, G + 1, 64], fp32, name="t1", tag="t1")
                        t2 = p1t.tile([128, G + 1, 64], fp32, name="t2", tag="t2")
                        nc.vector.tensor_tensor(t1[:], qkn[:, :, 0:64], csg, MUL)
                        nc.vector.tensor_tensor(t2[:], qkn[:, :, 64:128], sng, MUL)
                        nc.vector.tensor_tensor(qkr[:, :, 0:64], t1[:], t2[:], SUB)
                        nc.vector.tensor_tensor(t1[:], qkn[:, :, 64:128], csg, MUL)
                        nc.vector.tensor_tensor(t2[:], qkn[:, :, 0:64], sng, MUL)
                        nc.vector.tensor_tensor(qkr[:, :, 64:128], t1[:], t2[:], ADD)

                        # V: plain copy/cast into [s, h'] layout
                        nc.vector.tensor_copy(V_sb[:, tci, :], psB[:, H:2 * H])

                        col = tci * 128
                        for g in range(G):
                            pending.append(
                                (qkr[:, g, :], QT_sb[:, g, col:col + 128])
                            )
                        pending.append((qkr[:, G, :], KT_sb[:, col:col + 128]))
                flush_pending()

            # ================= Phase 2: attention =================
            with (
                tc.tile_pool(name="p2e", bufs=14) as p2e,
                tc.tile_pool(name="p2t", bufs=3) as p2t,
                tc.tile_pool(name="p3w", bufs=1) as p3w,
                tc.tile_pool(name="p3o", bufs=4) as p3o,
                tc.tile_pool(name="bcd", bufs=4, space="DRAM") as bcd,
            ):
                wo_sb = p3w.tile([128, G, D], bf16, name="wo_sb")
                for k in range(2):
                    nc.sync.dma_start(
                        out=wo_sb[:, 2 * k:2 * (k + 1), :],
                        in_=wo_d[:, 2 * k:2 * (k + 1), :],
                    )

                def emit_p3_half(tci, half, alt):
                    tcol = tci * 128
                    pso = []
                    for dh in range(4 * half, 4 * half + 4):
                        p = ps.tile([128, 512], fp32, name="pso", tag="ps")
                        pso.append((dh, p))
                    for g in range(G):
                        for dh, p in pso:
                            nc.tensor.matmul(
                                p[:],
                                OT_sb[:, g, tcol:tcol + 128],
                                wo_sb[:, g, dh * 512:(dh + 1) * 512],
                                start=(g == 0), stop=(g == G - 1),
                            )
                    for j, (dh, p) in enumerate(pso):
                        ob = p3o.tile([128, 512], fp32, name="ob", tag="ob")
                        if alt is None or (alt + j) % 2 != 0:
                            # interleaved region: keep DVE free for the
                            # mask/norm chain the AV matmuls wait on
                            nc.scalar.copy(ob[:], p[:])
                        else:
                            nc.vector.tensor_copy(ob[:], p[:])
                        nc.sync.dma_start(
                            out=out_d[tcol:tcol + 128, dh * 512:(dh + 1) * 512],
                            in_=ob[:],
                        )

                p3_queue = [
                    (tci, half) for tci in range(NTC) for half in range(2)
                ]
                p3_done = 0

                for b in range(B):
                    for g in range(G):
                        # pipeline both t0 halves: emit all 12 logits+exp
                        # first so the ACT exp stream runs ahead of the AVs
                        all_eTs = []
                        for t0 in (0, 512):
                            col0 = b * T + t0
                            nS = (t0 + 512) // 128
                            eTs = []
                            for c in range(nS):
                                # columns t_local < 128c - t0 are fully masked
                                # by causality — skip computing them entirely
                                off = max(0, 128 * c - t0)
                                Lp = ps.tile(
                                    [128, 512], fp32, name="Lp", tag="ps"
                                )
                                nc.tensor.matmul(
                                    Lp[:, off:512],
                                    KT_sb[:, b * T + c * 128:b * T + (c + 1) * 128],
                                    QT_sb[:, g, col0 + off:col0 + 512],
                                    start=True, stop=True,
                                )
                                eT = p2e.tile(
                                    [128, 512], bf16, name="eT", tag="eT"
                                )
                                nc.scalar.activation(
                                    eT[:, off:512], Lp[:, off:512],
                                    AF.Exp, scale=inv_sqrt_h,
                                )
                                if 128 * c + 127 > t0:  # diagonal-crossing block
                                    u0 = 512 + t0 - 128 * c
                                    nc.vector.tensor_tensor(
                                        eT[:, off:512], eT[:, off:512],
                                        mask_sb[:, u0 + off:u0 + 512], MUL,
                                    )
                                eTs.append((eT, off))
                            all_eTs.append((eTs, col0, nS))

                        # batch-0 o_proj chunks go HERE — between the logits
                        # and the AVs — exactly where the PE would otherwise
                        # wait for the exp stream to catch up
                        if b == 1:
                            for _ in range(4):
                                tci_h = p3_queue[p3_done]
                                emit_p3_half(tci_h[0], tci_h[1], None)
                                p3_done += 1

                        for eTs, col0, nS in all_eTs:
                            t0b = col0 - b * T
                            OTp = ps.tile([128, 512], fp32, name="OTp", tag="ps")
                            Sp = ps.tile([128, 512], fp32, name="Sp", tag="ps")
                            for c in range(nS):
                                sc = b * (T // 128) + c
                                eT, off = eTs[c]
                                nc.tensor.matmul(
                                    OTp[:, off:512], V_sb[:, sc, :],
                                    eT[:, off:512],
                                    start=(c == 0), stop=(c == nS - 1),
                                )
                                nc.tensor.matmul(
                                    Sp[:1, off:512], ones_bf[:], eT[:, off:512],
                                    start=(c == 0), stop=(c == nS - 1),
                                )
                            recip = rows.tile(
                                [1, 512], fp32, name="recip", tag="recip"
                            )
                            nc.vector.reciprocal(recip[:], Sp[:1, :])
                            # softmax normalization with the PE fully out of
                            # the path: bounce the recip row through DRAM and
                            # DMA it back partition-broadcast (DRAM-source
                            # zero-step partition APs are legal, cf. the
                            # groupnorm bias broadcast)
                            bd = bcd.tile([1, 512], fp32, name="bd", tag="bd")
                            nc.sync.dma_start(out=bd[:], in_=recip[:])
                            bcs = p2t.tile([128, 512], fp32, name="bcs", tag="bcs")
                            nc.gpsimd.dma_start(
                                out=bcs[:], in_=bd[:].to_broadcast((128, 512))
                            )
                            nc.vector.tensor_tensor(
                                OT_sb[:, g, col0:col0 + 512], OTp[:], bcs[:], MUL
                            )

                # ================= Phase 3: remaining o_proj =================
                while p3_done < len(p3_queue):
                    tci_h = p3_queue[p3_done]
                    emit_p3_half(tci_h[0], tci_h[1], p3_done)
                    p3_done += 1

    nc.compile()
    return nc


def _prep_inputs(x, wq, wk, wv, wo, q_scale, k_scale, segment_ids):
    """Host-side shard prep. Returns in_maps for the 8 cores."""
    x2 = np.ascontiguousarray(np.asarray(x, dtype=np.float32).reshape(BT, D))
    xT = x2.T.astype(BF)                                   # [D, BT]
    xq = np.ascontiguousarray(
        xT.reshape(DC, 128, NQ, 512).transpose(2, 1, 0, 3)
    )                                                      # [NQ,128,DC,512]

    seg = np.asarray(segment_ids)
    first = np.argmax(seg, axis=1)
    pos = np.where(
        seg != 0, np.arange(T, dtype=np.int64)[None, :] - first[:, None], 2 ** 30
    )
    fraction = np.arange(0, H, 2, dtype=np.float64) / H
    inv_freq = 1.0 / (ROPE_THETA ** fraction)
    sinus = pos.reshape(-1).astype(np.float64)[:, None] * inv_freq[None, :]
    cosf = np.cos(sinus).astype(np.float32)                # [BT, 64]
    sinf = np.sin(sinus).astype(np.float32)
    cosq = np.ascontiguousarray(cosf.reshape(NTC, 128, 64).transpose(1, 0, 2))
    sinq = np.ascontiguousarray(sinf.reshape(NTC, 128, 64).transpose(1, 0, 2))

    qsc = np.ascontiguousarray(
        np.tile(np.asarray(q_scale, np.float32)[None, :], (128, 1))
    )
    ksc = np.ascontiguousarray(
        np.tile(np.asarray(k_scale, np.float32)[None, :], (128, 1))
    )

    su = np.arange(128)[:, None] <= (np.arange(1024)[None, :] - 512)
    maskT = su.astype(BF)                                  # [128, 1024]

    wq2 = np.asarray(wq, np.float32).reshape(D, N * H)
    wk2 = np.asarray(wk, np.float32).reshape(D, KH * H)
    wv2 = np.asarray(wv, np.float32).reshape(D, KH * H)
    wo2 = np.asarray(wo, np.float32)                       # [N, H, D]

    in_maps = []
    for c in range(NCORES):
        wqkv = np.concatenate(
            [
                wq2[:, c * E:(c + 1) * E],
                wk2[:, c * H:(c + 1) * H],
                wv2[:, c * H:(c + 1) * H],
            ],
            axis=1,
        ).astype(BF)                                       # [D, 768]
        wqkvt = np.ascontiguousarray(
            wqkv.reshape(DC, 128, E + 2 * H).transpose(1, 0, 2)
        )                                                  # [128, DC, 768]
        woc = wo2[c * G:(c + 1) * G].astype(BF)            # [G, H, D]
        wot = np.ascontiguousarray(woc.transpose(1, 0, 2))  # [128, G, D]
        in_maps.append(
            {
                "xq": xq,
                "wqkv": wqkvt,
                "wo": wot,
                "cosq": cosq,
                "sinq": sinq,
                "qscale": qsc,
                "kscale": ksc,
                "maskT": maskT,
            }
        )
    return in_maps


def kernel(x, wq, wk, wv, wo, q_scale, k_scale, k_cache, v_cache,
           segment_ids, num_right_pads=0, **_unused):
    from concourse.bass_utils import run_bass_kernel_spmd

    if "nc" not in _CACHE:
        _CACHE["nc"] = _build()
    nc = _CACHE["nc"]

    in_maps = _prep_inputs(x, wq, wk, wv, wo, q_scale, k_scale, segment_ids)
    res = run_bass_kernel_spmd(nc, in_maps, core_ids=list(range(NCORES)))
    total = np.zeros((BT, D), np.float32)
    for c in range(NCORES):
        total += np.asarray(res.results[c]["out"], dtype=np.float32)
    return total.reshape(B, T, D)
